# revision 1
# baseline (speedup 1.0000x reference)
"""KNN classify kernel for TRN2 (8 NeuronCores).

Strategy: shard X over N (12500 points/core, padded to 12800). Every core
scores all 2048 queries against its shard with a single fused fp32r matmul
(scores s[b,n] = 2*q.x - ||x||^2; the per-row -||q||^2 term is dropped as it
doesn't change per-row ranking, and ||x||^2 enters as 3 extra bf16-split
contraction rows so no elementwise epilogue is needed). The DVE max/max_index
ops extract the top-8 candidates per 2560-wide slab. The host merges the
8x40 candidate lists per query, rescores a small rescue set exactly in fp64,
and emits the label-vote output.
"""

import sys

sys.path.insert(0, "/opt/trn_rl_repo")

import ml_dtypes
import numpy as np

import concourse.bacc as bacc
import concourse.mybir as mybir
from concourse import bass_utils
from concourse.tile import TileContext

B, D, N = 2048, 512, 100000
NCORES = 8
NSH = N // NCORES  # 12500 shard points per core
NPAD = 12800  # 25 * 512
NF = 512  # matmul moving free dim
SLAB = 2560  # top-8 scan window (5 chunks of 512)
NSLABS = NPAD // SLAB  # 5
NCH = SLAB // NF  # 5
P = 128
KC = 5  # contraction chunks: 512 data rows + 3 x2 rows + pad -> 640
KROWS = KC * P
BLK = B // P  # 16
NCAND = NSLABS * 8  # 40 candidates per core per query

_prog = None


def _build_program():
    nc = bacc.Bacc("TRN2", target_bir_lowering=False, debug=False, num_devices=NCORES)
    qt_d = nc.dram_tensor("qt", (KROWS, B), mybir.dt.float32r, kind="ExternalInput")
    xt_d = nc.dram_tensor("xt", (KROWS, NPAD), mybir.dt.float32r, kind="ExternalInput")
    vals_d = nc.dram_tensor("cand_vals", (B, NCAND), mybir.dt.float32, kind="ExternalOutput")
    idx_d = nc.dram_tensor("cand_idx", (B, NCAND), mybir.dt.uint32, kind="ExternalOutput")

    with TileContext(nc) as tc:
        with (
            tc.tile_pool(name="const", bufs=1) as cpool,
            tc.tile_pool(name="xtp", bufs=2) as xpool,
            tc.tile_pool(name="scp", bufs=3) as spool,
            tc.tile_pool(name="psp", bufs=8, space="PSUM") as ppool,
        ):
            # Separate tiles per block / per d-chunk: Tile deps are
            # tile-granular, so this is what lets the first matmuls start
            # after ~1.6 MB of DMA instead of ~12 MB (HAM cold-start fix).
            def load_qt_blk(blk):
                t = cpool.tile([P, KC, P], mybir.dt.float32r, tag=f"qt{blk}", name=f"qt{blk}")
                nc.sync.dma_start(
                    t,
                    qt_d.ap()[:, blk * P : (blk + 1) * P].rearrange(
                        "(c p) b -> p c b", p=P
                    ),
                )
                return t

            def load_xt_chunk(s, d):
                t = xpool.tile([P, SLAB], mybir.dt.float32r, tag=f"xt{d}", name=f"xt{s}_{d}")
                nc.sync.dma_start(
                    t,
                    xt_d.ap()[
                        d * P : (d + 1) * P, s * SLAB : (s + 1) * SLAB
                    ].rearrange("(c p) n -> p c n", p=P),
                )
                return t

            # Warm-up: dummy matmuls with no DMA deps run during the initial
            # ~14us input-DMA wait, so HAM un-throttles before real work.
            warm = cpool.tile([P, NF], mybir.dt.float32, tag="warm", name="warm")
            nc.vector.memset(warm, 0.0)
            wps = ppool.tile([P, NF], mybir.dt.float32, tag="ps", name="wps")
            for _ in range(30):
                nc.tensor.matmul(
                    wps[:, :P], warm[:, :P], warm[:, :P], start=True, stop=True
                )

            qts = [load_qt_blk(0)]
            cv = cpool.tile([P, BLK, NCAND], mybir.dt.float32, tag="cv")
            ci = cpool.tile([P, BLK, NCAND], mybir.dt.uint32, tag="ci")

            xts = [load_xt_chunk(0, d) for d in range(KC)]
            qts += [load_qt_blk(blk) for blk in range(1, BLK)]

            for s in range(NSLABS):
                xt = xts
                if s + 1 < NSLABS:
                    xts = []
                for blk in range(BLK):
                    sc = spool.tile([P, SLAB], mybir.dt.float32, tag="sc")
                    pss = [
                        ppool.tile([P, NF], mybir.dt.float32, tag="ps", name=f"ps{n}")
                        for n in range(NCH)
                    ]
                    for d in range(KC):
                        for n in range(NCH):
                            nc.tensor.matmul(
                                pss[n],
                                qts[blk][:, d, :],
                                xt[d][:, n * NF : (n + 1) * NF],
                                start=(d == 0),
                                stop=(d == KC - 1),
                            )
                    for n in range(NCH):
                        nc.scalar.copy(sc[:, n * NF : (n + 1) * NF], pss[n])
                    mv = cv[:, blk, s * 8 : (s + 1) * 8]
                    nc.vector.max(out=mv, in_=sc)
                    nc.vector.max_index(
                        out=ci[:, blk, s * 8 : (s + 1) * 8], in_max=mv, in_values=sc
                    )
                    # prefetch next slab's chunks spread across early blocks
                    if s + 1 < NSLABS and blk < KC:
                        xts.append(load_xt_chunk(s + 1, blk))

            nc.sync.dma_start(vals_d.ap().rearrange("(blk p) j -> p blk j", p=P), cv)
            nc.sync.dma_start(idx_d.ap().rearrange("(blk p) j -> p blk j", p=P), ci)

    nc.compile()
    return nc


def _prepare_inputs(queries, X):
    queries = np.asarray(queries, np.float32)
    X = np.asarray(X, np.float32)
    qt = np.zeros((KROWS, B), np.float32)
    qt[:D] = 2.0 * queries.T
    qt[D : D + 3] = 1.0

    x2 = (X.astype(np.float64) ** 2).sum(1)
    v = -x2
    p1 = v.astype(ml_dtypes.bfloat16).astype(np.float64)
    p2 = (v - p1).astype(ml_dtypes.bfloat16).astype(np.float64)
    p3 = (v - p1 - p2).astype(np.float32)

    Xt = X.T  # [D, N]
    in_maps = []
    for c in range(NCORES):
        sl = slice(c * NSH, (c + 1) * NSH)
        xt = np.zeros((KROWS, NPAD), np.float32)
        xt[:D, :NSH] = Xt[:, sl]
        xt[D, :NSH] = p1[sl].astype(np.float32)
        xt[D + 1, :NSH] = p2[sl].astype(np.float32)
        xt[D + 2, :NSH] = p3[sl]
        xt[D, NSH:] = -1e30  # padding columns always lose
        in_maps.append({"qt": qt, "xt": xt})
    return in_maps


def _run_device(queries, X, trace=False, trace_kwargs=None):
    global _prog
    if _prog is None:
        _prog = _build_program()
    in_maps = _prepare_inputs(queries, X)
    res = bass_utils.run_bass_kernel_spmd(
        _prog,
        in_maps,
        core_ids=list(range(NCORES)),
        trace=trace,
        **(trace_kwargs or {}),
    )
    return res


def _merge(queries, X, Y, K, res):
    vals = np.stack([res.results[c]["cand_vals"] for c in range(NCORES)])  # [8,B,40]
    idxs = np.stack([res.results[c]["cand_idx"] for c in range(NCORES)]).astype(
        np.int64
    )
    slab_off = (np.arange(NCAND) // 8) * SLAB
    gidx = idxs + slab_off[None, None, :] + (np.arange(NCORES) * NSH)[:, None, None]

    av = vals.transpose(1, 0, 2).reshape(B, NCORES * NCAND)
    ag = gidx.transpose(1, 0, 2).reshape(B, NCORES * NCAND)

    K = int(K)
    rescue = min(max(16, K), NCORES * NCAND)
    sel = np.argpartition(-av, rescue - 1, axis=1)[:, :rescue]
    cand = np.take_along_axis(ag, sel, 1)
    cand = np.clip(cand, 0, N - 1)

    qs = np.asarray(queries, np.float64)
    Xc = np.asarray(X, np.float64)[cand.reshape(-1)].reshape(B, rescue, D)
    d2 = ((Xc - qs[:, None, :]) ** 2).sum(-1)  # [B, rescue]
    order = np.argsort(d2, axis=1, kind="stable")[:, :K]
    top = np.take_along_axis(cand, order, 1)  # [B, K]

    labels = np.asarray(Y)[top].astype(np.float32)
    votes = labels.mean(1)
    out = np.zeros((B, 2), np.float32)
    out[:, 0] = votes
    return out


def kernel(queries, X, Y, K):
    res = _run_device(queries, X)
    return _merge(queries, X, Y, K, res)



# revision 4
# speedup vs baseline: 1.3825x; 1.3825x over previous
"""KNN classify kernel for TRN2 (8 NeuronCores).

Strategy: shard X over N (12500 points/core, padded to 12800). Scores
s[b,n] = 2*q.x + (R - ||x||^2) are computed with fp8e4 DoubleRow matmuls
(2 passes of K_eff=256 over the 512 data dims, plus one K=1 DoubleRow pass
carrying R-||x||^2 split into two e4m3 rows). The constant R and the
dropped -||q||^2 term don't affect per-row ranking. PSUM score tiles are
drained to SBUF as fp16 (mostly by the Scalar engine, some by DVE), then a
DVE pairwise-max fold tree pools each 2048-wide slab to 128 window maxima
(windows = columns congruent mod 128). MAX8/FIND_INDEX8 extract the top-8
windows per slab. The host expands the top windows (16 columns each),
rescores them exactly in fp64, takes top-K, and emits the label votes.
"""

import sys

sys.path.insert(0, "/opt/trn_rl_repo")

import ml_dtypes
import numpy as np

import concourse.bacc as bacc
import concourse.mybir as mybir
from concourse import bass_utils
from concourse.tile import TileContext

B, D, N = 2048, 512, 100000
NCORES = 8
NSH = N // NCORES  # 12500
NPAD = 12800  # 25 * 512
P = 128
BLK = B // P  # 16 query blocks
NF = 512  # psum bank width (fp32 cols)
SLAB = 2048  # 4 psum banks
NFULL = 6  # full slabs; + 1 partial slab of 512
WPART = NPAD - NFULL * SLAB  # 512
NSLABS = NFULL + 1
RW = 16  # window size (columns per pooled window)
NW = NSLABS * 8  # 56 candidate windows per (core, query)
GB = 4  # query-block group size for batched folds
RGLOB = 512.0  # global offset so R - ||x||^2 fits fp8e4

E4 = ml_dtypes.float8_e4m3
TOPW = 28  # windows expanded+rescored on host per query

_prog = None


def _build_program():
    nc = bacc.Bacc("TRN2", target_bir_lowering=False, debug=False, num_devices=NCORES)
    qt_d = nc.dram_tensor("qt", (BLK, P, 2, 2, P), mybir.dt.float8e4, kind="ExternalInput")
    xt_d = nc.dram_tensor("xt", (2, P, 2, NPAD), mybir.dt.float8e4, kind="ExternalInput")
    rx_d = nc.dram_tensor("rx", (1, 2, NPAD), mybir.dt.float8e4, kind="ExternalInput")
    vals_d = nc.dram_tensor("cand_vals", (B, NW), mybir.dt.float16, kind="ExternalOutput")
    idx_d = nc.dram_tensor("cand_idx", (B, NW), mybir.dt.uint16, kind="ExternalOutput")

    DR = mybir.MatmulPerfMode.DoubleRow
    MAX = mybir.AluOpType.max

    with TileContext(nc) as tc:
        with (
            tc.tile_pool(name="const", bufs=1) as cpool,
            tc.tile_pool(name="scp", bufs=3) as scpool,
            tc.tile_pool(name="fp", bufs=2) as fpool,
            tc.tile_pool(name="psp", bufs=2, space="PSUM") as ppool,
        ):
            def load_qt(blk):
                t = cpool.tile([P, 2, 2, P], mybir.dt.float8e4, tag=f"qt{blk}", name=f"qt{blk}")
                nc.sync.dma_start(t, qt_d.ap()[blk])
                return t

            def load_xt(c, s):
                w = SLAB if s < NFULL else WPART
                t = cpool.tile([P, 2, w], mybir.dt.float8e4, tag=f"xt{c}_{s}", name=f"xt{c}_{s}")
                nc.sync.dma_start(t, xt_d.ap()[c][:, :, s * SLAB : s * SLAB + w])
                return t

            qts = [load_qt(b) for b in range(BLK)]
            xts = {}
            for s in range(NSLABS):
                for c in (0, 1):
                    xts[(c, s)] = load_xt(c, s)
            rx_t = cpool.tile([1, 2, NPAD], mybir.dt.float8e4, tag="rx", name="rx")
            nc.sync.dma_start(rx_t, rx_d.ap())
            rq = cpool.tile([1, 2, P], mybir.dt.float8e4, tag="rq", name="rq")
            nc.vector.memset(rq, 1.0)

            warm = cpool.tile([P, P], mybir.dt.float32, tag="warm", name="warm")
            nc.vector.memset(warm, 0.0)
            cv = cpool.tile([P, BLK, NW], mybir.dt.float16, tag="cv")
            ci = cpool.tile([P, BLK, NW], mybir.dt.uint16, tag="ci")

            # Warm-up: dummy matmuls with no DMA deps run during the initial
            # input-DMA wait so HAM un-throttles before real work.
            wps = ppool.tile([P, 4, NF], mybir.dt.float32, tag="ps", name="wps")
            for _ in range(30):
                nc.tensor.matmul(wps[:, 0, :P], warm, warm, start=True, stop=True)

            drain_ctr = 0
            for s in range(NSLABS):
                w = SLAB if s < NFULL else WPART
                nch = w // NF
                sct = "scg" if s < NFULL else "scgp"
                for g in range(BLK // GB):
                    scg = scpool.tile([P, GB, nch, NF], mybir.dt.float16, tag=sct)
                    for b in range(GB):
                        blk = g * GB + b
                        ps = ppool.tile([P, 4, NF], mybir.dt.float32, tag="ps", name=f"ps{s}_{blk}")
                        for cpass in (0, 1):
                            for ch in range(nch):
                                nc.tensor.matmul(
                                    ps[:, ch, :],
                                    qts[blk][:, cpass],
                                    xts[(cpass, s)][:, :, ch * NF : (ch + 1) * NF],
                                    start=(cpass == 0),
                                    stop=False,
                                    perf_mode=DR,
                                )
                        for ch in range(nch):
                            nc.tensor.matmul(
                                ps[:, ch, :],
                                rq,
                                rx_t[:, :, s * SLAB + ch * NF : s * SLAB + (ch + 1) * NF],
                                start=False,
                                stop=True,
                                perf_mode=DR,
                            )
                        dst = scg[:, b]
                        src = ps[:, :nch, :]
                        if drain_ctr % 8 == 7:
                            nc.vector.tensor_copy(out=dst, in_=src)
                        else:
                            nc.scalar.copy(dst, src)
                        drain_ctr += 1

                    # fold tree: pool columns mod (w//16) across the group
                    if s < NFULL:
                        f1 = fpool.tile([P, GB, 2, NF], mybir.dt.float16, tag="f1")
                        nc.vector.tensor_tensor(
                            out=f1, in0=scg[:, :, :2, :], in1=scg[:, :, 2:, :], op=MAX
                        )
                        f2 = fpool.tile([P, GB, NF], mybir.dt.float16, tag="f2")
                        nc.vector.tensor_tensor(
                            out=f2, in0=f1[:, :, 0, :], in1=f1[:, :, 1, :], op=MAX
                        )
                        prev, pw = f2, NF
                    else:
                        prev, pw = scg[:, :, 0, :], NF
                    lvl = 0
                    while pw > w // RW:
                        pw //= 2
                        nxt = fpool.tile([P, GB, pw], mybir.dt.float16, tag=f"f{sct}{lvl}")
                        nc.vector.tensor_tensor(
                            out=nxt, in0=prev[:, :, :pw], in1=prev[:, :, pw:], op=MAX
                        )
                        prev = nxt
                        lvl += 1
                    f4 = prev
                    for b in range(GB):
                        blk = g * GB + b
                        mv = cv[:, blk, s * 8 : (s + 1) * 8]
                        nc.vector.max(out=mv, in_=f4[:, b, :])
                        nc.vector.max_index(
                            out=ci[:, blk, s * 8 : (s + 1) * 8], in_max=mv, in_values=f4[:, b, :]
                        )

            nc.sync.dma_start(vals_d.ap().rearrange("(blk p) j -> p blk j", p=P), cv)
            nc.sync.dma_start(idx_d.ap().rearrange("(blk p) j -> p blk j", p=P), ci)

    nc.compile()
    return nc


def _q8(a):
    return np.clip(a, -240.0, 240.0).astype(E4)


def _prepare_inputs(queries, X):
    queries = np.asarray(queries, np.float32)
    X = np.asarray(X, np.float32)

    q8 = _q8(2.0 * queries)  # [B, D]
    # qt[blk, p, c, i, m] = q8[blk*128+m, c*256+i*128+p]
    qt = np.ascontiguousarray(
        q8.reshape(BLK, P, 2, 2, P).transpose(0, 4, 2, 3, 1)
    )

    in_maps = []
    for core in range(NCORES):
        sl = slice(core * NSH, (core + 1) * NSH)
        Xc = np.zeros((NPAD, D), np.float32)
        Xc[:NSH] = X[sl]
        x8 = _q8(Xc)
        # xt[c, p, i, n] = x8[n, c*256+i*128+p]
        xt = np.ascontiguousarray(x8.reshape(NPAD, 2, 2, P).transpose(1, 3, 2, 0))

        x2 = (X[sl].astype(np.float64) ** 2).sum(1)
        t = np.full(NPAD, np.nan)
        t[:NSH] = RGLOB - x2
        hi = np.zeros(NPAD, E4)
        lo = np.zeros(NPAD, E4)
        hi[:NSH] = _q8(t[:NSH])
        lo[:NSH] = _q8(t[:NSH] - hi[:NSH].astype(np.float64))
        hi[NSH:] = E4(-240.0)
        lo[NSH:] = E4(-240.0)
        rx = np.stack([hi, lo])[None]  # [1, 2, NPAD]
        in_maps.append({"qt": qt, "xt": xt, "rx": np.ascontiguousarray(rx)})
    return in_maps


def _run_device(queries, X, trace=False, trace_kwargs=None):
    global _prog
    if _prog is None:
        _prog = _build_program()
    in_maps = _prepare_inputs(queries, X)
    res = bass_utils.run_bass_kernel_spmd(
        _prog,
        in_maps,
        core_ids=list(range(NCORES)),
        trace=trace,
        **(trace_kwargs or {}),
    )
    return res


def _merge(queries, X, Y, K, res):
    vals = np.stack([res.results[c]["cand_vals"] for c in range(NCORES)])  # [8,B,56]
    idxs = np.stack([res.results[c]["cand_idx"] for c in range(NCORES)]).astype(np.int64)

    # window (core, slab j//8, w) -> columns core*12500 + base + w + (w_step)*k
    slab_of = np.arange(NW) // 8  # [56]
    base = np.where(slab_of < NFULL, slab_of * SLAB, NFULL * SLAB)
    nwin = np.where(slab_of < NFULL, SLAB // RW, WPART // RW)  # 128 or 32

    av = vals.transpose(1, 0, 2).reshape(B, NCORES * NW).astype(np.float32)
    aw = idxs.transpose(1, 0, 2).reshape(B, NCORES * NW)
    wbase = np.tile(base, NCORES)[None, :] + np.repeat(np.arange(NCORES), NW)[None, :] * NSH
    wstep = np.tile(nwin, NCORES)[None, :]

    K = int(K)
    sel = np.argpartition(-av, TOPW - 1, axis=1)[:, :TOPW]  # [B, TOPW]
    selbase = np.take_along_axis(wbase + aw, sel, 1)  # [B, TOPW] first col of window
    selstep = np.take_along_axis(np.broadcast_to(wstep, aw.shape), sel, 1)
    # expand windows: col = base + step*k, k=0..15
    cand = selbase[:, :, None] + selstep[:, :, None] * np.arange(RW)[None, None, :]
    cand = cand.reshape(B, TOPW * RW)  # [B, 448]
    core_of = np.take_along_axis(
        np.broadcast_to(np.repeat(np.arange(NCORES), NW)[None, :], aw.shape), sel, 1
    )
    local = cand - np.repeat(core_of, RW, axis=1) * NSH
    invalid = local >= NSH
    cand = np.where(invalid, 0, cand)

    qs = np.asarray(queries, np.float64)
    Xf = np.asarray(X, np.float64)
    CB = 128
    top = np.empty((B, K), np.int64)
    for i in range(0, B, CB):
        j = min(i + CB, B)
        Xc = Xf[cand[i:j].reshape(-1)].reshape(j - i, -1, D)
        d2 = ((Xc - qs[i:j, None, :]) ** 2).sum(-1)
        d2 += invalid[i:j] * 1e30
        # dedupe duplicate columns (same col can appear via different windows? no,
        # windows are disjoint; but padding remaps to col 0) -> handled by invalid
        order = np.argsort(d2, axis=1, kind="stable")[:, :K]
        top[i:j] = np.take_along_axis(cand[i:j], order, 1)

    labels = np.asarray(Y)[top].astype(np.float32)
    votes = labels.mean(1)
    out = np.zeros((B, 2), np.float32)
    out[:, 0] = votes
    return out


def kernel(queries, X, Y, K):
    res = _run_device(queries, X)
    return _merge(queries, X, Y, K, res)


# revision 6
# speedup vs baseline: 1.8835x; 1.3624x over previous
"""KNN classify kernel for TRN2 (8 NeuronCores).

Strategy: shard X over N (12500 points/core, padded to 12800). Scores are
computed as s[b,n] = 2*q.x with two fp8e4 DoubleRow matmuls (K_eff=256 each
over the 512 dims). The -||x||^2 term is handled structurally: the host
sorts each core's points by ||x||^2 and permutes columns so that each
pooling window (16 columns congruent mod 128 within a 2048-wide slab) holds
16 norm-consecutive points, dealt round-robin across slabs. PSUM tiles are
drained to SBUF fp16 (Scalar engine mostly, DVE for a fraction), folded by
a DVE pairwise-max tree to 128 window maxima per slab, then a per-window
constant c[w] = min ||x||^2 (an fp16 input) is subtracted so window values
approximate max(2qx - x^2) with error bounded by the within-window norm
spread. MAX8/FIND_INDEX8 extract the top-8 windows per slab; the host
expands the top windows (16 columns each), rescores exactly in fp64, takes
top-K and emits label votes.
"""

import sys

sys.path.insert(0, "/opt/trn_rl_repo")

import ml_dtypes
import numpy as np

import concourse.bacc as bacc
import concourse.mybir as mybir
from concourse import bass_utils
from concourse.tile import TileContext

B, D, N = 2048, 512, 100000
NCORES = 8
NSH = N // NCORES  # 12500
NPAD = 12800  # 25 * 512
P = 128
BLK = B // P  # 16 query blocks
NF = 512  # psum bank width (fp32 cols)
SLAB = 2048  # 4 psum banks
NFULL = 6  # full slabs; + 1 partial slab of 512
WPART = NPAD - NFULL * SLAB  # 512
NSLABS = NFULL + 1
RW = 16  # columns per pooled window
NWIN = SLAB // RW  # 128 windows per full slab
NWINP = WPART // RW  # 32 windows in partial slab
NFWIN = NFULL * NWIN  # 768 windows live in full slabs; the rest go to the partial
NW = NSLABS * 8  # 56 candidates per (core, query)
GB = 4  # query-block group size for batched folds
DVE_DRAIN_EVERY = 16  # every Nth slab-drain goes to DVE instead of Scalar

E4 = ml_dtypes.float8_e4m3
TOPW = 40  # windows expanded+rescored on host per query

_prog = None


def _build_program():
    nc = bacc.Bacc("TRN2", target_bir_lowering=False, debug=False, num_devices=NCORES)
    qt_d = nc.dram_tensor("qt", (BLK, P, 2, 2, P), mybir.dt.float8e4, kind="ExternalInput")
    xt_d = nc.dram_tensor("xt", (2, P, 2, NPAD), mybir.dt.float8e4, kind="ExternalInput")
    c_d = nc.dram_tensor("cwin", (P, NSLABS, NWIN), mybir.dt.float16, kind="ExternalInput")
    vals_d = nc.dram_tensor("cand_vals", (B, NW), mybir.dt.float16, kind="ExternalOutput")
    idx_d = nc.dram_tensor("cand_idx", (B, NW), mybir.dt.uint16, kind="ExternalOutput")

    DR = mybir.MatmulPerfMode.DoubleRow
    MAX = mybir.AluOpType.max
    SUB = mybir.AluOpType.subtract

    with TileContext(nc) as tc:
        with (
            tc.tile_pool(name="const", bufs=1) as cpool,
            tc.tile_pool(name="scp", bufs=3) as scpool,
            tc.tile_pool(name="fp", bufs=2) as fpool,
            tc.tile_pool(name="psp", bufs=2, space="PSUM") as ppool,
        ):
            def load_qt(blk):
                t = cpool.tile([P, 2, 2, P], mybir.dt.float8e4, tag=f"qt{blk}", name=f"qt{blk}")
                nc.sync.dma_start(t, qt_d.ap()[blk])
                return t

            def load_xt(c, s):
                w = SLAB if s < NFULL else WPART
                t = cpool.tile([P, 2, w], mybir.dt.float8e4, tag=f"xt{c}_{s}", name=f"xt{c}_{s}")
                nc.sync.dma_start(t, xt_d.ap()[c][:, :, s * SLAB : s * SLAB + w])
                return t

            qts = [load_qt(b) for b in range(BLK)]
            xts = {}
            for s in range(NSLABS):
                for c in (0, 1):
                    xts[(c, s)] = load_xt(c, s)
            c_t = cpool.tile([P, NSLABS, NWIN], mybir.dt.float16, tag="cwin", name="cwin")
            nc.sync.dma_start(c_t, c_d.ap())

            warm = cpool.tile([P, P], mybir.dt.float32, tag="warm", name="warm")
            nc.vector.memset(warm, 0.0)
            cv = cpool.tile([P, BLK, NW], mybir.dt.float16, tag="cv")
            ci = cpool.tile([P, BLK, NW], mybir.dt.uint16, tag="ci")

            # Warm-up: dummy matmuls with no DMA deps run during the initial
            # input-DMA wait so HAM un-throttles before real work.
            wps = ppool.tile([P, 4, NF], mybir.dt.float32, tag="ps", name="wps")
            for _ in range(100):
                nc.tensor.matmul(wps[:, 0, :P], warm, warm, start=True, stop=True)

            drain_ctr = 0
            for s in range(NSLABS):
                w = SLAB if s < NFULL else WPART
                nch = w // NF
                nwin = w // RW
                sct = "scg" if s < NFULL else "scgp"
                for g in range(BLK // GB):
                    scg = scpool.tile([P, GB, nch, NF], mybir.dt.float16, tag=sct)
                    for b in range(GB):
                        blk = g * GB + b
                        ps = ppool.tile([P, 4, NF], mybir.dt.float32, tag="ps", name=f"ps{s}_{blk}")
                        for cpass in (0, 1):
                            for ch in range(nch):
                                nc.tensor.matmul(
                                    ps[:, ch, :],
                                    qts[blk][:, cpass],
                                    xts[(cpass, s)][:, :, ch * NF : (ch + 1) * NF],
                                    start=(cpass == 0),
                                    stop=(cpass == 1),
                                    perf_mode=DR,
                                )
                        dst = scg[:, b]
                        src = ps[:, :nch, :]
                        if drain_ctr % DVE_DRAIN_EVERY == DVE_DRAIN_EVERY - 1:
                            nc.vector.tensor_copy(out=dst, in_=src)
                        else:
                            nc.scalar.copy(dst, src)
                        drain_ctr += 1

                    # fold tree: pool columns mod (w//16) across the group
                    if s < NFULL:
                        f1 = fpool.tile([P, GB, 2, NF], mybir.dt.float16, tag="f1")
                        nc.vector.tensor_tensor(
                            out=f1, in0=scg[:, :, :2, :], in1=scg[:, :, 2:, :], op=MAX
                        )
                        f2 = fpool.tile([P, GB, NF], mybir.dt.float16, tag="f2")
                        nc.vector.tensor_tensor(
                            out=f2, in0=f1[:, :, 0, :], in1=f1[:, :, 1, :], op=MAX
                        )
                        prev, pw = f2, NF
                    else:
                        prev, pw = scg[:, :, 0, :], NF
                    lvl = 0
                    while pw > nwin:
                        pw //= 2
                        nxt = fpool.tile([P, GB, pw], mybir.dt.float16, tag=f"f{sct}{lvl}")
                        nc.vector.tensor_tensor(
                            out=nxt, in0=prev[:, :, :pw], in1=prev[:, :, pw:], op=MAX
                        )
                        prev = nxt
                        lvl += 1
                    cs = fpool.tile([P, GB, nwin], mybir.dt.float16, tag=f"cs{sct}")
                    for b in range(GB):
                        nc.vector.tensor_tensor(
                            out=cs[:, b], in0=prev[:, b], in1=c_t[:, s, :nwin], op=SUB
                        )
                    for b in range(GB):
                        blk = g * GB + b
                        mv = cv[:, blk, s * 8 : (s + 1) * 8]
                        nc.vector.max(out=mv, in_=cs[:, b])
                        nc.vector.max_index(
                            out=ci[:, blk, s * 8 : (s + 1) * 8], in_max=mv, in_values=cs[:, b]
                        )

            nc.sync.dma_start(vals_d.ap().rearrange("(blk p) j -> p blk j", p=P), cv)
            nc.sync.dma_start(idx_d.ap().rearrange("(blk p) j -> p blk j", p=P), ci)

    nc.compile()
    return nc


def _q8(a):
    return np.clip(a, -240.0, 240.0).astype(E4)


def _permutation():
    """sorted-rank r -> device column, for one core (NSH points).

    Window w = r//16 (16 norm-consecutive points); full-slab windows are
    dealt round-robin across the 6 full slabs; the remainder go to the
    partial slab. Returns dev_col[r]."""
    r = np.arange(NSH)
    wnd = r // RW
    j = r % RW
    full = wnd < NFWIN
    s = wnd % NFULL
    k = wnd // NFULL
    col_full = s * SLAB + j * NWIN + k
    pk = wnd - NFWIN
    col_part = NFULL * SLAB + j * NWINP + pk
    return np.where(full, col_full, col_part)


def _prepare_inputs(queries, X):
    queries = np.asarray(queries, np.float32)
    X = np.asarray(X, np.float32)

    q8 = _q8(2.0 * queries)  # [B, D]
    # qt[blk, p, c, i, m] = q8[blk*128+m, c*256+i*128+p]
    qt = np.ascontiguousarray(q8.reshape(BLK, P, 2, 2, P).transpose(0, 4, 2, 3, 1))

    dev_col = _permutation()  # [NSH]
    in_maps = []
    orig_maps = []
    for core in range(NCORES):
        sl = slice(core * NSH, (core + 1) * NSH)
        Xc = X[sl]
        x2 = (Xc.astype(np.float64) ** 2).sum(1)
        order = np.argsort(x2, kind="stable")  # ascending norm

        Xdev = np.zeros((NPAD, D), np.float32)
        Xdev[dev_col] = Xc[order]
        orig_of_col = np.full(NPAD, -1, np.int64)
        orig_of_col[dev_col] = core * NSH + order
        orig_maps.append(orig_of_col)

        x8 = _q8(Xdev)
        # xt[c, p, i, n] = x8[n, c*256+i*128+p]
        xt = np.ascontiguousarray(x8.reshape(NPAD, 2, 2, P).transpose(1, 3, 2, 0))

        # c[s, k] = min ||x||^2 of window (s, k); +30000 for empty windows
        cw = np.full((NSLABS, NWIN), 30000.0, np.float64)
        x2s = x2[order]
        wnd = np.arange(NSH) // RW
        wmin = np.minimum.reduceat(x2s, np.arange(0, NSH, RW))  # [NSH//16.. ceil]
        nwnd = wmin.shape[0]
        wi = np.arange(nwnd)
        full = wi < NFWIN
        cw[wi[full] % NFULL, wi[full] // NFULL] = wmin[full]
        cw[NFULL, wi[~full] - NFWIN] = wmin[~full]
        cwin = np.broadcast_to(cw.astype(np.float16)[None], (P, NSLABS, NWIN))
        in_maps.append({"qt": qt, "xt": xt, "cwin": np.ascontiguousarray(cwin)})
    return in_maps, orig_maps


def _run_device(queries, X, trace=False, trace_kwargs=None):
    global _prog
    if _prog is None:
        _prog = _build_program()
    in_maps, orig_maps = _prepare_inputs(queries, X)
    res = bass_utils.run_bass_kernel_spmd(
        _prog,
        in_maps,
        core_ids=list(range(NCORES)),
        trace=trace,
        **(trace_kwargs or {}),
    )
    res.orig_maps = orig_maps
    return res


def _merge(queries, X, Y, K, res):
    vals = np.stack([res.results[c]["cand_vals"] for c in range(NCORES)])  # [8,B,56]
    idxs = np.stack([res.results[c]["cand_idx"] for c in range(NCORES)]).astype(np.int64)
    orig = np.stack(res.orig_maps)  # [8, NPAD] original X row per device col, -1 pad

    slab_of = np.arange(NW) // 8  # [56]
    base = np.where(slab_of < NFULL, slab_of * SLAB, NFULL * SLAB)
    step = np.where(slab_of < NFULL, NWIN, NWINP)

    av = vals.transpose(1, 0, 2).reshape(B, NCORES * NW).astype(np.float32)
    aw = idxs.transpose(1, 0, 2).reshape(B, NCORES * NW)
    col0 = np.tile(base, NCORES)[None, :] + aw  # first col of window (core-local)
    wstep = np.tile(step, NCORES)[None, :]
    core_of = np.repeat(np.arange(NCORES), NW)[None, :]

    K = int(K)
    sel = np.argpartition(-av, TOPW - 1, axis=1)[:, :TOPW]  # [B, TOPW]
    selc0 = np.take_along_axis(np.broadcast_to(col0, av.shape).copy(), sel, 1)
    selst = np.take_along_axis(np.broadcast_to(wstep, av.shape).copy(), sel, 1)
    selco = np.take_along_axis(np.broadcast_to(core_of, av.shape).copy(), sel, 1)
    cols = selc0[:, :, None] + selst[:, :, None] * np.arange(RW)[None, None, :]
    cols = cols.reshape(B, TOPW * RW)
    cores = np.repeat(selco, RW, axis=1)
    cand = orig[cores, cols]  # [B, TOPW*RW] original X row or -1
    invalid = cand < 0
    cand = np.where(invalid, 0, cand)

    qs = np.asarray(queries, np.float64)
    Xf = np.asarray(X, np.float64)
    CB = 64
    top = np.empty((B, K), np.int64)
    for i in range(0, B, CB):
        j = min(i + CB, B)
        Xc = Xf[cand[i:j].reshape(-1)].reshape(j - i, -1, D)
        d2 = ((Xc - qs[i:j, None, :]) ** 2).sum(-1)
        d2 += invalid[i:j] * 1e30
        order = np.argsort(d2, axis=1, kind="stable")[:, :K]
        top[i:j] = np.take_along_axis(cand[i:j], order, 1)

    labels = np.asarray(Y)[top].astype(np.float32)
    votes = labels.mean(1)
    out = np.zeros((B, 2), np.float32)
    out[:, 0] = votes
    return out


def kernel(queries, X, Y, K):
    res = _run_device(queries, X)
    return _merge(queries, X, Y, K, res)


# revision 9
# speedup vs baseline: 1.9665x; 1.0441x over previous
"""KNN classify kernel for TRN2 (8 NeuronCores).

Strategy: shard X over N (12500 points/core, padded to 12800). Scores are
computed as s[b,n] = 2*q.x with two fp8e4 DoubleRow matmuls (K_eff=256 each
over the 512 dims). The -||x||^2 term is handled structurally: the host
sorts each core's points by ||x||^2 and permutes columns so that each
pooling window (16 columns congruent mod 128 within a 2048-wide slab) holds
16 norm-consecutive points, dealt round-robin across slabs. PSUM tiles are
drained to SBUF fp16 (Scalar engine mostly, DVE every 8th), folded by a DVE
pairwise-max tree to 128 window maxima per slab, then a per-window constant
c[w] = min ||x||^2 (an fp16 input) is subtracted so window values
approximate max(2qx - x^2) with error bounded by the within-window norm
spread. MAX8/FIND_INDEX8 extract the top-8 windows per slab-pair; the host
expands the top windows (16 columns each), rescores exactly in fp64, takes
top-K and emits label votes.
"""

import sys

sys.path.insert(0, "/opt/trn_rl_repo")

import ml_dtypes
import numpy as np

import concourse.bacc as bacc
import concourse.mybir as mybir
from concourse import bass_utils
from concourse.tile import TileContext

B, D, N = 2048, 512, 100000
NCORES = 8
NSH = N // NCORES  # 12500
NPAD = 12800  # 25 * 512
P = 128
BLK = B // P  # 16 query blocks
NF = 512  # psum bank width (fp32 cols)
SLAB = 2048  # 4 psum banks
NFULL = 6  # full slabs; + 1 partial slab of 512
WPART = NPAD - NFULL * SLAB  # 512
NSLABS = NFULL + 1
RW = 16  # columns per pooled window
NWIN = SLAB // RW  # 128 windows per full slab
NWINP = WPART // RW  # 32 windows in partial slab
NFWIN = NFULL * NWIN  # 768 windows live in full slabs; the rest go to the partial
NPAIR = NFULL // 2  # slab pairs sharing one top-8 extraction
NW = (NPAIR + 1) * 8  # 32 candidates per (core, query)
GB = 4  # query-block group size for batched folds
DVE_DRAIN_EVERY = 8  # every Nth slab-drain goes to DVE instead of Scalar

E4 = ml_dtypes.float8_e4m3
TOPW = 40  # windows expanded+rescored on host per query

_prog = None


def _build_program():
    nc = bacc.Bacc("TRN2", target_bir_lowering=False, debug=False, num_devices=NCORES)
    qt_d = nc.dram_tensor("qt", (BLK, P, 2, 2, P), mybir.dt.float8e4, kind="ExternalInput")
    xt_d = nc.dram_tensor("xt", (2, P, 2, NPAD), mybir.dt.float8e4, kind="ExternalInput")
    c_d = nc.dram_tensor("cwin", (P, NSLABS, GB, NWIN), mybir.dt.float16, kind="ExternalInput")
    vals_d = nc.dram_tensor("cand_vals", (B, NW), mybir.dt.float16, kind="ExternalOutput")
    idx_d = nc.dram_tensor("cand_idx", (B, NW), mybir.dt.uint16, kind="ExternalOutput")

    DR = mybir.MatmulPerfMode.DoubleRow
    MAX = mybir.AluOpType.max
    SUB = mybir.AluOpType.subtract

    with TileContext(nc) as tc:
        with (
            tc.tile_pool(name="const", bufs=1) as cpool,
            tc.tile_pool(name="scp", bufs=4) as scpool,
            tc.tile_pool(name="fp", bufs=2) as fpool,
            tc.tile_pool(name="psp", bufs=2, space="PSUM") as ppool,
        ):
            def load_qt(blk):
                t = cpool.tile([P, 2, 2, P], mybir.dt.float8e4, tag=f"qt{blk}", name=f"qt{blk}")
                nc.sync.dma_start(t, qt_d.ap()[blk])
                return t

            def load_xt(c, s):
                w = SLAB if s < NFULL else WPART
                t = cpool.tile([P, 2, w], mybir.dt.float8e4, tag=f"xt{c}_{s}", name=f"xt{c}_{s}")
                nc.sync.dma_start(t, xt_d.ap()[c][:, :, s * SLAB : s * SLAB + w])
                return t

            # first slab's inputs first so the PE can start ASAP
            qts = [load_qt(0)]
            xts = {(c, 0): load_xt(c, 0) for c in (0, 1)}
            qts += [load_qt(b) for b in range(1, BLK)]
            for s in range(1, NSLABS):
                for c in (0, 1):
                    xts[(c, s)] = load_xt(c, s)
            c_t = cpool.tile([P, NSLABS, GB, NWIN], mybir.dt.float16, tag="cwin", name="cwin")
            nc.sync.dma_start(c_t, c_d.ap())

            warm = cpool.tile([P, P], mybir.dt.float32, tag="warm", name="warm")
            nc.vector.memset(warm, 0.0)
            cv = cpool.tile([P, BLK, NW], mybir.dt.float16, tag="cv")
            ci = cpool.tile([P, BLK, NW], mybir.dt.uint16, tag="ci")

            # Warm-up: dummy matmuls with no DMA deps run during the initial
            # input-DMA wait so HAM un-throttles before real work.
            wps = ppool.tile([P, 4, NF], mybir.dt.float32, tag="ps", name="wps")
            for _ in range(40):
                nc.tensor.matmul(wps[:, 0, :P], warm, warm, start=True, stop=True)

            cs_tiles = {}
            drain_ctr = 0
            for s in range(NSLABS):
                w = SLAB if s < NFULL else WPART
                nch = w // NF
                nwin = w // RW
                sct = "scg" if s < NFULL else "scgp"
                for g in range(BLK // GB):
                    scg = scpool.tile([P, GB, nch, NF], mybir.dt.float16, tag=sct)
                    for b in range(GB):
                        blk = g * GB + b
                        ps = ppool.tile([P, 4, NF], mybir.dt.float32, tag="ps", name=f"ps{s}_{blk}")
                        for cpass in (0, 1):
                            for ch in range(nch):
                                nc.tensor.matmul(
                                    ps[:, ch, :],
                                    qts[blk][:, cpass],
                                    xts[(cpass, s)][:, :, ch * NF : (ch + 1) * NF],
                                    start=(cpass == 0),
                                    stop=(cpass == 1),
                                    perf_mode=DR,
                                )
                        dst = scg[:, b]
                        src = ps[:, :nch, :]
                        if drain_ctr % DVE_DRAIN_EVERY == DVE_DRAIN_EVERY - 1:
                            nc.vector.tensor_copy(out=dst, in_=src)
                        else:
                            nc.scalar.copy(dst, src)
                        drain_ctr += 1

                    # fold tree: pool columns mod (w//16) across the group
                    if s < NFULL:
                        f1 = fpool.tile([P, GB, 2, NF], mybir.dt.float16, tag="f1")
                        nc.vector.tensor_tensor(
                            out=f1, in0=scg[:, :, :2, :], in1=scg[:, :, 2:, :], op=MAX
                        )
                        f2 = fpool.tile([P, GB, NF], mybir.dt.float16, tag="f2")
                        nc.vector.tensor_tensor(
                            out=f2, in0=f1[:, :, 0, :], in1=f1[:, :, 1, :], op=MAX
                        )
                        prev, pw = f2, NF
                    else:
                        prev, pw = scg[:, :, 0, :], NF
                    lvl = 0
                    while pw > nwin:
                        pw //= 2
                        nxt = fpool.tile([P, GB, pw], mybir.dt.float16, tag=f"f{sct}{lvl}")
                        nc.vector.tensor_tensor(
                            out=nxt, in0=prev[:, :, :pw], in1=prev[:, :, pw:], op=MAX
                        )
                        prev = nxt
                        lvl += 1
                    if s < NFULL:
                        # c-subtract into the slab-pair extraction tile
                        if s % 2 == 0:
                            cs_tiles[g] = fpool.tile(
                                [P, GB, 2 * NWIN],
                                mybir.dt.float16,
                                tag=f"cs{g}",
                                name=f"cs{g}_{s}",
                            )
                        cs = cs_tiles[g]
                        nc.vector.tensor_tensor(
                            out=cs[:, :, (s % 2) * NWIN : (s % 2 + 1) * NWIN],
                            in0=prev,
                            in1=c_t[:, s],
                            op=SUB,
                        )
                        if s % 2 == 1:
                            pair = s // 2
                            for b in range(GB):
                                blk = g * GB + b
                                mv = cv[:, blk, pair * 8 : (pair + 1) * 8]
                                nc.vector.max(out=mv, in_=cs[:, b])
                                nc.vector.max_index(
                                    out=ci[:, blk, pair * 8 : (pair + 1) * 8],
                                    in_max=mv,
                                    in_values=cs[:, b],
                                )
                    else:
                        csp = fpool.tile([P, GB, NWINP], mybir.dt.float16, tag="csp")
                        nc.vector.tensor_tensor(
                            out=csp, in0=prev, in1=c_t[:, s, :, :NWINP], op=SUB
                        )
                        for b in range(GB):
                            blk = g * GB + b
                            mv = cv[:, blk, NPAIR * 8 : (NPAIR + 1) * 8]
                            nc.vector.max(out=mv, in_=csp[:, b])
                            nc.vector.max_index(
                                out=ci[:, blk, NPAIR * 8 : (NPAIR + 1) * 8],
                                in_max=mv,
                                in_values=csp[:, b],
                            )

            nc.sync.dma_start(vals_d.ap().rearrange("(blk p) j -> p blk j", p=P), cv)
            nc.sync.dma_start(idx_d.ap().rearrange("(blk p) j -> p blk j", p=P), ci)

    nc.compile()
    return nc


def _q8(a):
    return np.clip(a, -240.0, 240.0).astype(E4)


def _permutation():
    """sorted-rank r -> device column, for one core (NSH points).

    Window w = r//16 (16 norm-consecutive points); full-slab windows are
    dealt round-robin across the 6 full slabs; the remainder go to the
    partial slab. Returns dev_col[r]."""
    r = np.arange(NSH)
    wnd = r // RW
    j = r % RW
    full = wnd < NFWIN
    s = wnd % NFULL
    k = wnd // NFULL
    col_full = s * SLAB + j * NWIN + k
    pk = wnd - NFWIN
    col_part = NFULL * SLAB + j * NWINP + pk
    return np.where(full, col_full, col_part)


def _prepare_inputs(queries, X):
    queries = np.asarray(queries, np.float32)
    X = np.asarray(X, np.float32)

    q8 = _q8(2.0 * queries)  # [B, D]
    # qt[blk, p, c, i, m] = q8[blk*128+m, c*256+i*128+p]
    qt = np.ascontiguousarray(q8.reshape(BLK, P, 2, 2, P).transpose(0, 4, 2, 3, 1))

    dev_col = _permutation()  # [NSH]
    in_maps = []
    orig_maps = []
    for core in range(NCORES):
        sl = slice(core * NSH, (core + 1) * NSH)
        Xc = X[sl]
        x2 = (Xc.astype(np.float64) ** 2).sum(1)
        order = np.argsort(x2, kind="stable")  # ascending norm

        Xdev = np.zeros((NPAD, D), np.float32)
        Xdev[dev_col] = Xc[order]
        orig_of_col = np.full(NPAD, -1, np.int64)
        orig_of_col[dev_col] = core * NSH + order
        orig_maps.append(orig_of_col)

        x8 = _q8(Xdev)
        # xt[c, p, i, n] = x8[n, c*256+i*128+p]
        xt = np.ascontiguousarray(x8.reshape(NPAD, 2, 2, P).transpose(1, 3, 2, 0))

        # c[s, k] = min ||x||^2 of window (s, k); +30000 for empty windows
        cw = np.full((NSLABS, NWIN), 30000.0, np.float64)
        x2s = x2[order]
        wmin = np.minimum.reduceat(x2s, np.arange(0, NSH, RW))
        nwnd = wmin.shape[0]
        wi = np.arange(nwnd)
        full = wi < NFWIN
        cw[wi[full] % NFULL, wi[full] // NFULL] = wmin[full]
        cw[NFULL, wi[~full] - NFWIN] = wmin[~full]
        cwin = np.broadcast_to(
            cw.astype(np.float16)[None, :, None, :], (P, NSLABS, GB, NWIN)
        )
        in_maps.append({"qt": qt, "xt": xt, "cwin": np.ascontiguousarray(cwin)})
    return in_maps, orig_maps


def _run_device(queries, X, trace=False, trace_kwargs=None):
    global _prog
    if _prog is None:
        _prog = _build_program()
    in_maps, orig_maps = _prepare_inputs(queries, X)
    res = bass_utils.run_bass_kernel_spmd(
        _prog,
        in_maps,
        core_ids=list(range(NCORES)),
        trace=trace,
        **(trace_kwargs or {}),
    )
    res.orig_maps = orig_maps
    return res


def _merge(queries, X, Y, K, res):
    vals = np.stack([res.results[c]["cand_vals"] for c in range(NCORES)])  # [8,B,32]
    idxs = np.stack([res.results[c]["cand_idx"] for c in range(NCORES)]).astype(np.int64)
    orig = np.stack(res.orig_maps)  # [8, NPAD] original X row per device col, -1 pad

    # slot j: pair p=j//8 (<NPAIR: slabs 2p,2p+1, idx in [0,256)); else partial
    pair_of = np.arange(NW) // 8
    is_part = pair_of >= NPAIR

    av = vals.transpose(1, 0, 2).reshape(B, NCORES * NW).astype(np.float32)
    aw = idxs.transpose(1, 0, 2).reshape(B, NCORES * NW)
    pf = np.tile(pair_of, NCORES)[None, :]
    pp = np.tile(is_part, NCORES)[None, :]
    # decode window -> (first col, step) in core-local device columns
    slab = np.where(pp, NFULL, 2 * pf + (aw >= NWIN))
    wloc = np.where(pp, aw, aw % NWIN)
    col0 = slab * SLAB + wloc
    wstep = np.where(pp, NWINP, NWIN)
    core_of = np.repeat(np.arange(NCORES), NW)[None, :]

    K = int(K)
    sel = np.argpartition(-av, TOPW - 1, axis=1)[:, :TOPW]  # [B, TOPW]
    selc0 = np.take_along_axis(col0, sel, 1)
    selst = np.take_along_axis(np.broadcast_to(wstep, av.shape), sel, 1)
    selco = np.take_along_axis(np.broadcast_to(core_of, av.shape), sel, 1)
    cols = selc0[:, :, None] + selst[:, :, None] * np.arange(RW)[None, None, :]
    cols = cols.reshape(B, TOPW * RW)
    cores = np.repeat(selco, RW, axis=1)
    cand = orig[cores, cols]  # [B, TOPW*RW] original X row or -1
    invalid = cand < 0
    cand = np.where(invalid, 0, cand)

    qs = np.asarray(queries, np.float64)
    Xf = np.asarray(X, np.float64)
    CB = 64
    top = np.empty((B, K), np.int64)
    for i in range(0, B, CB):
        j = min(i + CB, B)
        Xc = Xf[cand[i:j].reshape(-1)].reshape(j - i, -1, D)
        d2 = ((Xc - qs[i:j, None, :]) ** 2).sum(-1)
        d2 += invalid[i:j] * 1e30
        order = np.argsort(d2, axis=1, kind="stable")[:, :K]
        top[i:j] = np.take_along_axis(cand[i:j], order, 1)

    labels = np.asarray(Y)[top].astype(np.float32)
    votes = labels.mean(1)
    out = np.zeros((B, 2), np.float32)
    out[:, 0] = votes
    return out


def kernel(queries, X, Y, K):
    res = _run_device(queries, X)
    return _merge(queries, X, Y, K, res)
